# revision 8
# baseline (speedup 1.0000x reference)
"""Series decomposition: depthwise moving-average (box filter, W=25, replicate
padding) + remainder, data-parallel over batch across 8 NeuronCores.

Per core: x shard [4, 512, 4096] viewed as [2048, 4096] rows. For each
[128, 4096] tile:

  ACT   xs = x * (1/W) in fp16 (3 ops: center + both replicate pads)
  DVE   init = sum(xs[0:25]) (fp32), then one tensor_tensor_scan
            state = (xs[i+12] + state) - xs[i-13]
        emits trend directly in fp16 (state is fp32 internally; the same
        fp16 value is added and later subtracted, so the telescoping is
        exact), then remainder = x - trend via tensor_tensor subtract
        (all-fp16 operands keep the DVE in its 2x perf mode; the fused
        scalar_tensor_tensor form has NO fast mode and costs 2x more)
  DMA   trend out via the ACT HWDGE queue, x in + remainder out via SP

All HBM I/O is fp16 (inputs quantized on host, outputs upcast on host),
halving traffic vs fp32 (rel err ~1e-3 << 2e-2 budget). The kernel is
DMA-bound: in the timeline cost model its steady state exactly matches the
pure-DMA floor for this traffic (engines ~70% busy, DMA 100%).
"""

import numpy as np

import concourse.bacc as bacc
import concourse.bass as bass
import concourse.mybir as mybir
from concourse.bass_utils import run_bass_kernel_spmd
from concourse.tile import TileContext

B, C, L, W = 32, 512, 4096, 25
PAD = W // 2  # 12
NCORES = 8
ROWS = (B // NCORES) * C  # 2048 rows per core
P = 128
NTILES = ROWS // P  # 16

# SBUF tile layout (fp16): x lives at [LEFT, LEFT+L); 13 left-pad cols feed
# the scan's subtract lag, 12 right-pad cols feed its leading edge. LEFT=32
# keeps the big DMA-in destination 64B-aligned.
LEFT = 32
LPAD = PAD + 1  # 13
XW = LEFT + L + PAD + 4  # 4144

FP32 = mybir.dt.float32
F16 = mybir.dt.float16
DT_NP = np.float16  # host-side dtype of the kernel's HBM I/O


def build_nc(
    scale: float,
    rows: int = ROWS,
    l: int = L,
    repeats: int = 1,
    bufs: int = 5,
) -> bass.Bass:
    """repeats>1 re-runs the whole sweep inside one NEFF (timing harnesses
    use this to make device time dominate per-call dispatch overhead)."""
    ntiles = rows // P
    nc = bacc.Bacc(trn_type="TRN2")
    x = nc.dram_tensor("x", [rows, l], F16, kind="ExternalInput")
    trend = nc.dram_tensor("trend", [rows, l], F16, kind="ExternalOutput")
    remainder = nc.dram_tensor("remainder", [rows, l], F16, kind="ExternalOutput")

    with TileContext(nc) as tc:
        with tc.tile_pool(name="pool", bufs=bufs) as pool:
            for i in range(ntiles * repeats):
                i = i % ntiles
                rsl = slice(i * P, (i + 1) * P)
                xp = pool.tile([P, XW], F16, tag="xp")
                nc.sync.dma_start(out=xp[:, LEFT : LEFT + l], in_=x[rsl, :])
                # ACT: xs = x*scale, replicate-padded on both sides
                xs = pool.tile([P, XW], F16, tag="xs")
                nc.scalar.mul(xs[:, LEFT : LEFT + l], xp[:, LEFT : LEFT + l], scale)
                nc.scalar.mul(
                    xs[:, LEFT - LPAD : LEFT],
                    xp[:, LEFT : LEFT + 1].to_broadcast((P, LPAD)),
                    scale,
                )
                nc.scalar.mul(
                    xs[:, LEFT + l : LEFT + l + PAD],
                    xp[:, LEFT + l - 1 : LEFT + l].to_broadcast((P, PAD)),
                    scale,
                )
                # window sum at i=-1 plus the lagged element the first scan
                # step subtracts: sum of xs cols [-13..11]
                init = pool.tile([P, 1], FP32, tag="init")
                nc.vector.tensor_reduce(
                    out=init[:, 0:1],
                    in_=xs[:, LEFT - LPAD : LEFT - LPAD + W],
                    axis=mybir.AxisListType.X,
                    op=mybir.AluOpType.add,
                )
                t = pool.tile([P, l], F16, tag="t")
                nc.vector.tensor_tensor_scan(
                    out=t[:, :],
                    data0=xs[:, LEFT - LPAD + W : LEFT - LPAD + W + l],
                    data1=xs[:, LEFT - LPAD : LEFT - LPAD + l],
                    initial=init[:, 0:1],
                    op0=mybir.AluOpType.add,
                    op1=mybir.AluOpType.subtract,
                )
                r = pool.tile([P, l], F16, tag="r")
                nc.vector.tensor_tensor(
                    out=r[:, :],
                    in0=xp[:, LEFT : LEFT + l],
                    in1=t[:, :],
                    op=mybir.AluOpType.subtract,
                )
                nc.scalar.dma_start(out=trend[rsl, :], in_=t[:, :])
                nc.sync.dma_start(out=remainder[rsl, :], in_=r[:, :])
    nc.finalize()
    return nc


def _probe_devices():
    """Touch every NeuronCore with a trivial computation. After a previous
    client exits with in-flight bass executions, the first bass exec from a
    fresh client can fail with NRT_EXEC_UNIT_UNRECOVERABLE; a plain jax
    computation resets the state."""
    try:
        import jax
        import jax.numpy as jnp

        for d in jax.devices():
            y = jax.device_put(np.ones((4, 4), np.float32), d)
            jnp.sum(y).block_until_ready()
    except Exception:
        pass


def kernel(x, weight):
    x = np.asarray(x, dtype=np.float32)
    # frozen depthwise moving-average kernel: every tap is 1/W
    scale = float(np.asarray(weight).reshape(-1)[0])
    nc = build_nc(scale)
    shards = np.ascontiguousarray(x.astype(DT_NP)).reshape(NCORES, ROWS, L)
    in_maps = [{"x": shards[c]} for c in range(NCORES)]
    _probe_devices()
    out = None
    for attempt in range(3):
        try:
            out = run_bass_kernel_spmd(nc, in_maps, core_ids=list(range(NCORES)))
            break
        except Exception:
            if attempt == 2:
                raise
            # a dirty previous client session can leave the device mesh
            # "unrecoverable"; a fresh PJRT client + probe clears it
            try:
                import jax

                jax.clear_backends()
            except Exception:
                pass
            _probe_devices()
    trend = np.concatenate(
        [np.asarray(out.results[c]["trend"], dtype=np.float32)[None] for c in range(NCORES)],
        axis=0,
    ).reshape(B, C, L)
    remainder = np.concatenate(
        [np.asarray(out.results[c]["remainder"], dtype=np.float32)[None] for c in range(NCORES)],
        axis=0,
    ).reshape(B, C, L)
    return trend, remainder
